# revision 20
# baseline (speedup 1.0000x reference)
"""Multi-head attention on 8 Trainium2 NeuronCores.

Sharding: data-parallel over batch (2) x tensor-parallel over heads
(16 heads -> 4 per core). Each core computes QKV projections for its
4 heads, masked softmax attention, and a partial output projection
(row-parallel Wo); the host sums the 4 per-batch partials and folds
in the biases that commute with the linear output projection
(out[b] = sum_partials.T + bo + Wo @ bv).

Per-core pipeline (layouts chosen so no on-chip transposes are needed):
  - host sends query^T/key^T/value^T [D,S] fp16, per-core weight
    slices pre-transposed fp16, mask01^T fp16 (1=keep, 0=masked).
  - Q^T,K^T [256,S] = Wq/k^T.T @ x^T   (dout on partitions, fp16+FWL,
    bias added on ScalarE Identity-activation, fp16 out)
  - V [S,256] = x^T.T @ Wv^T           (tokens on partitions, fp16)
  - scores^T [k,q] = K^T_h.T @ Q^T_h   (two heads row-packed per PSUM tile)
  - p^T = exp(scores/8)                (ScalarE, fp16 out)
  - p^T *= mask01^T                    (DVE fp16 2x, broadcast across heads)
  - X'^T [65,q] = [V_h | 1].T @ p^T    (row 64 = softmax denominator)
  - drain X' to SBUF early (frees PSUM), then batched fast-reciprocal
    of the 4 denominators, GPSIMD partition-broadcast, DVE normalize.
  - out^T partial [D,S] = Wo^T.T @ X^T (fp16, f32 out, drained on ScalarE)
"""

import os
import numpy as np

B, S, D = 2, 2048, 1024
H, DK = 16, 64
NCORES = 8
CPB = 4                 # cores per batch
HPC = H // CPB          # 4 heads per core
HD = HPC * DK           # 256
P = 128
QG = 512                # query block
NQG = S // QG
NKB = S // P            # key blocks
NDC = D // P            # d_in chunks
SCALE = 1.0 / np.sqrt(DK)

_CACHE = {}


def _build_nc():
    import concourse.mybir as mybir
    import concourse.tile as tile
    from concourse import bacc

    f32 = mybir.dt.float32
    f16 = mybir.dt.float16
    AF = mybir.ActivationFunctionType
    OP = mybir.AluOpType

    nc = bacc.Bacc("TRN2", target_bir_lowering=False, debug=False)
    xqT = nc.dram_tensor("xqT", [D, S], f16, kind="ExternalInput")
    xkT = nc.dram_tensor("xkT", [D, S], f16, kind="ExternalInput")
    xvT = nc.dram_tensor("xvT", [D, S], f16, kind="ExternalInput")
    wqT = nc.dram_tensor("wqT", [D, HD], f16, kind="ExternalInput")
    wkT = nc.dram_tensor("wkT", [D, HD], f16, kind="ExternalInput")
    wvT = nc.dram_tensor("wvT", [D, HD], f16, kind="ExternalInput")
    woT = nc.dram_tensor("woT", [HD, D], f16, kind="ExternalInput")
    bq = nc.dram_tensor("bq", [HD, 1], f32, kind="ExternalInput")
    bk = nc.dram_tensor("bk", [HD, 1], f32, kind="ExternalInput")
    maskT = nc.dram_tensor("maskT", [S, S], f16, kind="ExternalInput")
    outT = nc.dram_tensor("outT", [D, S], f32, kind="ExternalOutput")

    with tile.TileContext(nc) as tc:
        with (
            tc.tile_pool(name="wpool", bufs=1) as wpool,
            tc.tile_pool(name="xkq", bufs=8) as xkq,
            tc.tile_pool(name="xqp", bufs=8) as xqp,
            tc.tile_pool(name="xvp", bufs=8) as xvp,
            tc.tile_pool(name="proj", bufs=1) as proj,
            tc.tile_pool(name="vpool", bufs=1) as vpool,
            tc.tile_pool(name="mpool", bufs=4) as mpool,
            tc.tile_pool(name="ppool", bufs=3) as ppool,
            tc.tile_pool(name="xspool", bufs=4) as xspool,
            tc.tile_pool(name="npool", bufs=2) as npool,
            tc.tile_pool(name="npool1", bufs=1) as npool1,
            tc.tile_pool(name="xnpool", bufs=1) as xnpool,
            tc.tile_pool(name="opool", bufs=4) as opool,
        ):
            # ---------------- weights -> SBUF ----------------
            wq_sb = wpool.tile([P, NDC * HD], f16, tag="wq")
            wk_sb = wpool.tile([P, NDC * HD], f16, tag="wk")
            wv_sb = wpool.tile([P, NDC * HD], f16, tag="wv")
            def _load_w(wsb, wdr):
                nc.sync.dma_start(
                    wsb[:].rearrange("p (c n) -> p c n", n=HD),
                    wdr.rearrange("(c p) n -> p c n", p=P),
                )
            _load_w(wk_sb, wkT)
            bq_sb = [wpool.tile([P, 1], f32, tag=f"bq{i}", name=f"bqt{i}")
                     for i in range(2)]
            bk_sb = [wpool.tile([P, 1], f32, tag=f"bk{i}", name=f"bkt{i}")
                     for i in range(2)]
            for i in range(2):
                nc.sync.dma_start(bq_sb[i][:], bq[i * P:(i + 1) * P, :])
                nc.sync.dma_start(bk_sb[i][:], bk[i * P:(i + 1) * P, :])
            wo_sb = [wpool.tile([P, D], f16, tag=f"wo{i}", name=f"wo{i}")
                     for i in range(2)]

            KT = [proj.tile([P, S], f16, tag=f"KT{i}", name=f"KT{i}")
                  for i in range(2)]
            QT = [proj.tile([P, S], f16, tag=f"QT{i}", name=f"QT{i}")
                  for i in range(2)]
            Vt = [vpool.tile([P, HPC * (DK + 1)], f16, tag=f"V{t}",
                             name=f"Vt{t}") for t in range(NKB)]

            with tc.tile_pool(name="pps", bufs=8, space="PSUM") as pps:
                # ---------------- K projection (c-outer, DMA-paced) ------
                xk = [xkq.tile([P, S], f16, tag="xin", name="xk")
                      for _ in range(NDC)]
                for c in range(NDC):
                    nc.sync.dma_start(xk[c][:], xkT[c * P:(c + 1) * P, :])
                ps_k = [pps.tile([P, QG], f32, tag="projps", name="pps")
                        for _ in range(2 * NQG)]
                for c in range(NDC):
                    for tg in range(NQG):
                        for kc in range(2):
                            nc.tensor.matmul(
                                ps_k[tg * 2 + kc][:],
                                wk_sb[:, c * HD + kc * P: c * HD + (kc + 1) * P],
                                xk[c][:, tg * QG:(tg + 1) * QG],
                                start=(c == 0), stop=(c == NDC - 1),
                            )
                for tg in range(NQG):
                    for kc in range(2):
                        nc.scalar.activation(
                            KT[kc][:, tg * QG:(tg + 1) * QG],
                            ps_k[tg * 2 + kc][:],
                            AF.Identity, bias=bk_sb[kc][:, 0:1],
                        )

                # ---------------- Q projection (c-outer) ----------------
                _load_w(wq_sb, wqT)
                xq = [xqp.tile([P, S], f16, tag="xq", name="xq")
                      for _ in range(NDC)]
                for c in range(NDC):
                    nc.sync.dma_start(xq[c][:], xqT[c * P:(c + 1) * P, :])
                ps_q = [pps.tile([P, QG], f32, tag="projps", name="pps")
                        for _ in range(2 * NQG)]
                for c in range(NDC):
                    for tg in range(NQG):
                        for kc in range(2):
                            nc.tensor.matmul(
                                ps_q[tg * 2 + kc][:],
                                wq_sb[:, c * HD + kc * P: c * HD + (kc + 1) * P],
                                xq[c][:, tg * QG:(tg + 1) * QG],
                                start=(c == 0), stop=(c == NDC - 1),
                            )
                for tg in range(NQG):
                    for kc in range(2):
                        nc.scalar.activation(
                            QT[kc][:, tg * QG:(tg + 1) * QG],
                            ps_q[tg * 2 + kc][:],
                            AF.Identity, bias=bq_sb[kc][:, 0:1],
                        )

            with tc.tile_pool(name="vps", bufs=2, space="PSUM") as vps:
                # ---------------- V projection ----------------
                _load_w(wv_sb, wvT)
                xv = [xvp.tile([P, S], f16, tag="xv", name="xv")
                      for _ in range(NDC)]
                for c in range(NDC):
                    nc.sync.dma_start(xv[c][:], xvT[c * P:(c + 1) * P, :])
                for i in range(2):
                    nc.sync.dma_start(wo_sb[i][:], woT[i * P:(i + 1) * P, :])
                for t in range(NKB):
                    vp = vps.tile([P, HD], f32, tag="vps", name="vps")
                    for c in range(NDC):
                        nc.tensor.matmul(
                            vp[:], xv[c][:, t * P:(t + 1) * P],
                            wv_sb[:, c * HD:(c + 1) * HD],
                            start=(c == 0), stop=(c == NDC - 1),
                        )
                    v5 = Vt[t][:].rearrange("p (h c) -> p h c", c=DK + 1)
                    nc.vector.tensor_copy(
                        v5[:, :, 0:DK],
                        vp[:].rearrange("p (h c) -> p h c", c=DK),
                    )
                    nc.gpsimd.memset(v5[:, :, DK:DK + 1], 1.0)

            XN = [xnpool.tile([P, S], f16, tag=f"XN{i}", name=f"XN{i}")
                  for i in range(2)]

            # ---------------- attention ----------------
            with tc.tile_pool(name="scps", bufs=2, space="PSUM") as scps, \
                 tc.tile_pool(name="pvps", bufs=1, space="PSUM") as pvps:
                def kb_front(qg, kb):
                    """mask DMA + scores + exp + mask-mul for one key block;
                    returns the two pa views (one per head pair)."""
                    mk = mpool.tile([P, QG], f16, tag="mk", name="mk")
                    nc.sync.dma_start(
                        mk[:],
                        maskT[kb * P:(kb + 1) * P, qg * QG:(qg + 1) * QG],
                    )
                    pa = ppool.tile([P, 4 * QG], f16, tag="pa", name="pa")
                    for pr in range(2):
                        sc = scps.tile([P, 2 * QG], f32, tag="sc", name="sc")
                        for hh in range(2):
                            h = pr * 2 + hh
                            kc, sub = divmod(h, 2)
                            nc.tensor.matmul(
                                sc[:, hh * QG:(hh + 1) * QG],
                                KT[kc][sub * DK:(sub + 1) * DK,
                                       kb * P:(kb + 1) * P],
                                QT[kc][sub * DK:(sub + 1) * DK,
                                       qg * QG:(qg + 1) * QG],
                                start=True, stop=True,
                                tile_position=(sub * DK, 0),
                            )
                        nc.scalar.activation(
                            pa[:, pr * 2 * QG:(pr + 1) * 2 * QG], sc[:],
                            AF.Exp, scale=float(SCALE))
                    pav4 = pa[:].rearrange("p (h n) -> p h n", n=QG)
                    nc.vector.tensor_tensor(
                        pav4, pav4,
                        mk[:].unsqueeze(1).broadcast_to((P, HPC, QG)),
                        op=OP.mult,
                    )
                    return pav4

                def kb_pv(pvt, kb, pav4):
                    for h in range(HPC):
                        nc.tensor.matmul(
                            pvt[h][:],
                            Vt[kb][:].rearrange(
                                "p (h c) -> p h c", c=DK + 1)[:, h, :],
                            pav4[:, h, :],
                            start=(kb == 0), stop=(kb == NKB - 1),
                        )

                def emit_outproj(qg, dm_lo=0, dm_hi=D // P):
                    for dm in range(dm_lo, dm_hi):
                        ops_t = pvps.tile([P, QG], f32, tag=f"pv{dm % HPC}",
                                          name=f"op{dm}")
                        for hd in range(2):
                            nc.tensor.matmul(
                                ops_t[:],
                                wo_sb[hd][:, dm * P:(dm + 1) * P],
                                XN[hd][:, qg * QG:(qg + 1) * QG],
                                start=(hd == 0), stop=(hd == 1),
                            )
                        ost = opool.tile([P, QG], f32, tag="ost", name="ost")
                        if dm % 2 == 0:
                            nc.scalar.activation(ost[:], ops_t[:], AF.Copy)
                        else:
                            nc.vector.tensor_copy(ost[:], ops_t[:])
                        nc.sync.dma_start(
                            outT[dm * P:(dm + 1) * P, qg * QG:(qg + 1) * QG],
                            ost[:])

                for qg in range(NQG):
                    # interleave the previous block's out-projection with this
                    # block's first two score/exp stages: exp fills the ACT
                    # bubble while outproj drains the PV psum slots (merged-pa
                    # keeps this within the 3-slot pa budget).
                    if qg > 0:
                        emit_outproj(qg - 1, 0, 2)
                    e0 = kb_front(qg, 0)
                    if qg > 0:
                        emit_outproj(qg - 1, 2, 5)
                    e1 = kb_front(qg, 1)
                    if qg > 0:
                        emit_outproj(qg - 1, 5, 8)
                    pvt = [pvps.tile([DK + 1, QG], f32, tag=f"pv{h}",
                                     name=f"pv{h}") for h in range(HPC)]
                    kb_pv(pvt, 0, e0)
                    kb_pv(pvt, 1, e1)
                    for kb in range(2, NKB):
                        kb_pv(pvt, kb, kb_front(qg, kb))
                    # drain PV psum early, then normalize off-critical-path
                    # (DVE partition-offset writes must be 32-aligned, so
                    # the 4 denominator rows are gathered at 32-stride.)
                    XS = []
                    dn4 = npool1.tile([3 * 32 + 1, QG], f32, tag="dn4",
                                      name="dn4")
                    for h in range(HPC):
                        xs = xspool.tile([DK + 1, QG], f32, tag="xs",
                                         name="xs")
                        nc.scalar.activation(xs[:], pvt[h][:], AF.Copy)
                        XS.append(xs)
                    for h in range(HPC):
                        nc.vector.tensor_copy(dn4[32 * h:32 * h + 1, :],
                                              XS[h][DK:DK + 1, :])
                    rc4 = npool1.tile([3 * 32 + 1, QG], f32, tag="rc4",
                                      name="rc4")
                    nc.vector.reciprocal_approx_fast(rc4[:], dn4[:])
                    for h in range(HPC):
                        kc, sub = divmod(h, 2)
                        r1 = npool.tile([1, QG], f32, tag="r1", name="r1")
                        nc.vector.tensor_copy(r1[:],
                                              rc4[32 * h:32 * h + 1, :])
                        rb = npool.tile([DK, QG], f32, tag="rb", name="rb")
                        nc.gpsimd.partition_broadcast(rb[:], r1[:],
                                                      channels=DK)
                        nc.vector.tensor_tensor(
                            XN[kc][sub * DK:(sub + 1) * DK,
                                   qg * QG:(qg + 1) * QG],
                            XS[h][0:DK, :], rb[:], op=OP.mult,
                        )
                emit_outproj(NQG - 1)

    nc.compile()
    return nc


def _get_nc():
    if "nc" not in _CACHE:
        _CACHE["nc"] = _build_nc()
    return _CACHE["nc"]


def _install_trace_shim():
    """Register the axon NTFF profile hook (dev/test only)."""
    import sys, types
    if "antenv.axon_hooks" in sys.modules:
        return
    try:
        import antenv
        from trn_agent_boot.trn_boot import _ntff_profile_via_ctypes
    except ImportError:
        return
    mod = types.ModuleType("antenv.axon_hooks")
    _hook = [_ntff_profile_via_ctypes("/opt/axon/libaxon_pjrt.so")]
    mod.get_axon_ntff_profile_hook = lambda: _hook[0]
    mod.set_axon_ntff_profile_hook = lambda h: _hook.__setitem__(0, h)
    sys.modules["antenv.axon_hooks"] = mod
    antenv.axon_hooks = mod


def kernel(query, key, value, mask, Wq, bq, Wk, bk, Wv, bv, Wo, bo):
    from concourse.bass_utils import run_bass_kernel_spmd

    query = np.asarray(query, np.float32)
    key = np.asarray(key, np.float32)
    value = np.asarray(value, np.float32)
    mask = np.asarray(mask)
    Wq = np.asarray(Wq, np.float32); bq = np.asarray(bq, np.float32)
    Wk = np.asarray(Wk, np.float32); bk = np.asarray(bk, np.float32)
    Wv = np.asarray(Wv, np.float32); bv = np.asarray(bv, np.float32)
    Wo = np.asarray(Wo, np.float32); bo = np.asarray(bo, np.float32)

    nc = _get_nc()

    qT = {b: np.ascontiguousarray(query[b].T.astype(np.float16))
          for b in range(B)}
    kT = {b: np.ascontiguousarray(key[b].T.astype(np.float16))
          for b in range(B)}
    vT = {b: np.ascontiguousarray(value[b].T.astype(np.float16))
          for b in range(B)}
    mT = {b: np.ascontiguousarray((mask[b].T == 0).astype(np.float16))
          for b in range(B)}

    in_maps = []
    for c in range(NCORES):
        b, hg = divmod(c, CPB)
        sl = slice(hg * HD, (hg + 1) * HD)
        in_maps.append({
            "xqT": qT[b],
            "xkT": kT[b],
            "xvT": vT[b],
            "wqT": np.ascontiguousarray(Wq[sl].T.astype(np.float16)),
            "wkT": np.ascontiguousarray(Wk[sl].T.astype(np.float16)),
            "wvT": np.ascontiguousarray(Wv[sl].T.astype(np.float16)),
            "woT": np.ascontiguousarray(Wo[:, sl].T.astype(np.float16)),
            "bq": np.ascontiguousarray(bq[sl].reshape(HD, 1)),
            "bk": np.ascontiguousarray(bk[sl].reshape(HD, 1)),
            "maskT": mT[b],
        })

    trace = bool(int(os.environ.get("BASS_KERNEL_TRACE", "0")))
    if trace:
        _install_trace_shim()
    res = run_bass_kernel_spmd(nc, in_maps, core_ids=list(range(NCORES)),
                               trace=trace)
    _CACHE["last_perf"] = res

    out = np.zeros((B, S, D), np.float32)
    for c in range(NCORES):
        b = c // CPB
        out[b] += res.results[c]["outT"].T
    out += (Wo @ bv + bo)[None, None, :]
    return out


# revision 21
# speedup vs baseline: 1.0343x; 1.0343x over previous
"""Multi-head attention on 8 Trainium2 NeuronCores.

Sharding: data-parallel over batch (2) x tensor-parallel over heads
(16 heads -> 4 per core). Each core computes QKV projections for its
4 heads, masked softmax attention, and a partial output projection
(row-parallel Wo); the host sums the 4 per-batch partials and folds
in the biases that commute with the linear output projection
(out[b] = sum_partials.T + bo + Wo @ bv).

Per-core pipeline (layouts chosen so no on-chip transposes are needed):
  - host sends query^T/key^T/value^T [D,S] fp16, per-core weight
    slices pre-transposed fp16, mask01^T fp16 (1=keep, 0=masked).
  - Q^T,K^T [256,S] = Wq/k^T.T @ x^T   (dout on partitions, fp16+FWL,
    bias added on ScalarE Identity-activation, fp16 out)
  - V [S,256] = x^T.T @ Wv^T           (tokens on partitions, fp16)
  - scores^T [k,q] = K^T_h.T @ Q^T_h   (two heads row-packed per PSUM tile)
  - p^T = exp(scores/8)                (ScalarE, fp16 out)
  - p^T *= mask01^T                    (DVE fp16 2x, broadcast across heads)
  - X'^T [65,q] = [V_h | 1].T @ p^T    (row 64 = softmax denominator)
  - drain X' to SBUF early (frees PSUM), then batched fast-reciprocal
    of the 4 denominators, GPSIMD partition-broadcast, DVE normalize.
  - out^T partial [D,S] = Wo^T.T @ X^T (fp16, f32 out, drained on ScalarE)
"""

import os
import numpy as np

B, S, D = 2, 2048, 1024
H, DK = 16, 64
NCORES = 8
CPB = 4                 # cores per batch
HPC = H // CPB          # 4 heads per core
HD = HPC * DK           # 256
P = 128
QG = 512                # query block
NQG = S // QG
NKB = S // P            # key blocks
NDC = D // P            # d_in chunks
SCALE = 1.0 / np.sqrt(DK)

_CACHE = {}


def _build_nc():
    import concourse.mybir as mybir
    import concourse.tile as tile
    from concourse import bacc

    f32 = mybir.dt.float32
    f16 = mybir.dt.float16
    AF = mybir.ActivationFunctionType
    OP = mybir.AluOpType

    nc = bacc.Bacc("TRN2", target_bir_lowering=False, debug=False)
    xqT = nc.dram_tensor("xqT", [D, S], f16, kind="ExternalInput")
    xkT = nc.dram_tensor("xkT", [D, S], f16, kind="ExternalInput")
    xvT = nc.dram_tensor("xvT", [D, S], f16, kind="ExternalInput")
    wqT = nc.dram_tensor("wqT", [D, HD], f16, kind="ExternalInput")
    wkT = nc.dram_tensor("wkT", [D, HD], f16, kind="ExternalInput")
    wvT = nc.dram_tensor("wvT", [D, HD], f16, kind="ExternalInput")
    woT = nc.dram_tensor("woT", [HD, D], f16, kind="ExternalInput")
    bq = nc.dram_tensor("bq", [HD, 1], f32, kind="ExternalInput")
    bk = nc.dram_tensor("bk", [HD, 1], f32, kind="ExternalInput")
    maskT = nc.dram_tensor("maskT", [S, S], f16, kind="ExternalInput")
    outT = nc.dram_tensor("outT", [D, S], f32, kind="ExternalOutput")

    with tile.TileContext(nc) as tc:
        with (
            tc.tile_pool(name="wpool", bufs=1) as wpool,
            tc.tile_pool(name="xkq", bufs=8) as xkq,
            tc.tile_pool(name="xqp", bufs=8) as xqp,
            tc.tile_pool(name="xvp", bufs=8) as xvp,
            tc.tile_pool(name="proj", bufs=1) as proj,
            tc.tile_pool(name="vpool", bufs=1) as vpool,
            tc.tile_pool(name="mpool", bufs=4) as mpool,
            tc.tile_pool(name="ppool", bufs=3) as ppool,
            tc.tile_pool(name="xspool", bufs=4) as xspool,
            tc.tile_pool(name="npool", bufs=2) as npool,
            tc.tile_pool(name="npool1", bufs=1) as npool1,
            tc.tile_pool(name="xnpool", bufs=1) as xnpool,
            tc.tile_pool(name="opool", bufs=4) as opool,
        ):
            # ---------------- weights -> SBUF ----------------
            wq_sb = wpool.tile([P, NDC * HD], f16, tag="wq")
            wk_sb = wpool.tile([P, NDC * HD], f16, tag="wk")
            wv_sb = wpool.tile([P, NDC * HD], f16, tag="wv")
            def _load_w(wsb, wdr):
                nc.sync.dma_start(
                    wsb[:].rearrange("p (c n) -> p c n", n=HD),
                    wdr.rearrange("(c p) n -> p c n", p=P),
                )
            _load_w(wk_sb, wkT)
            bq_sb = [wpool.tile([P, 1], f32, tag=f"bq{i}", name=f"bqt{i}")
                     for i in range(2)]
            bk_sb = [wpool.tile([P, 1], f32, tag=f"bk{i}", name=f"bkt{i}")
                     for i in range(2)]
            for i in range(2):
                nc.sync.dma_start(bq_sb[i][:], bq[i * P:(i + 1) * P, :])
                nc.sync.dma_start(bk_sb[i][:], bk[i * P:(i + 1) * P, :])
            wo_sb = [wpool.tile([P, D], f16, tag=f"wo{i}", name=f"wo{i}")
                     for i in range(2)]

            KT = [proj.tile([P, S], f16, tag=f"KT{i}", name=f"KT{i}")
                  for i in range(2)]
            QT = [proj.tile([P, S], f16, tag=f"QT{i}", name=f"QT{i}")
                  for i in range(2)]
            Vt = [vpool.tile([P, HPC * (DK + 1)], f16, tag=f"V{t}",
                             name=f"Vt{t}") for t in range(NKB)]

            with tc.tile_pool(name="pps", bufs=8, space="PSUM") as pps:
                # ---------------- K projection (c-outer, DMA-paced) ------
                xk = [xkq.tile([P, S], f16, tag="xin", name="xk")
                      for _ in range(NDC)]
                for c in range(NDC):
                    nc.sync.dma_start(xk[c][:], xkT[c * P:(c + 1) * P, :])
                ps_k = [pps.tile([P, QG], f32, tag="projps", name="pps")
                        for _ in range(2 * NQG)]
                for c in range(NDC):
                    for tg in range(NQG):
                        for kc in range(2):
                            nc.tensor.matmul(
                                ps_k[tg * 2 + kc][:],
                                wk_sb[:, c * HD + kc * P: c * HD + (kc + 1) * P],
                                xk[c][:, tg * QG:(tg + 1) * QG],
                                start=(c == 0), stop=(c == NDC - 1),
                            )
                for tg in range(NQG):
                    for kc in range(2):
                        nc.scalar.activation(
                            KT[kc][:, tg * QG:(tg + 1) * QG],
                            ps_k[tg * 2 + kc][:],
                            AF.Identity, bias=bk_sb[kc][:, 0:1],
                        )

                # ---------------- Q projection (c-outer) ----------------
                _load_w(wq_sb, wqT)
                xq = [xqp.tile([P, S], f16, tag="xq", name="xq")
                      for _ in range(NDC)]
                for c in range(NDC):
                    nc.sync.dma_start(xq[c][:], xqT[c * P:(c + 1) * P, :])
                ps_q = [pps.tile([P, QG], f32, tag="projps", name="pps")
                        for _ in range(2 * NQG)]
                for c in range(NDC):
                    for tg in range(NQG):
                        for kc in range(2):
                            nc.tensor.matmul(
                                ps_q[tg * 2 + kc][:],
                                wq_sb[:, c * HD + kc * P: c * HD + (kc + 1) * P],
                                xq[c][:, tg * QG:(tg + 1) * QG],
                                start=(c == 0), stop=(c == NDC - 1),
                            )
                for tg in range(NQG):
                    for kc in range(2):
                        nc.scalar.activation(
                            QT[kc][:, tg * QG:(tg + 1) * QG],
                            ps_q[tg * 2 + kc][:],
                            AF.Identity, bias=bq_sb[kc][:, 0:1],
                        )

            with tc.tile_pool(name="vps", bufs=2, space="PSUM") as vps:
                # ---------------- V projection ----------------
                _load_w(wv_sb, wvT)
                xv = [xvp.tile([P, S], f16, tag="xv", name="xv")
                      for _ in range(NDC)]
                for c in range(NDC):
                    nc.sync.dma_start(xv[c][:], xvT[c * P:(c + 1) * P, :])
                for i in range(2):
                    nc.sync.dma_start(wo_sb[i][:], woT[i * P:(i + 1) * P, :])
                for t in range(NKB):
                    vp = vps.tile([P, HD], f32, tag="vps", name="vps")
                    for c in range(NDC):
                        nc.tensor.matmul(
                            vp[:], xv[c][:, t * P:(t + 1) * P],
                            wv_sb[:, c * HD:(c + 1) * HD],
                            start=(c == 0), stop=(c == NDC - 1),
                        )
                    v5 = Vt[t][:].rearrange("p (h c) -> p h c", c=DK + 1)
                    nc.vector.tensor_copy(
                        v5[:, :, 0:DK],
                        vp[:].rearrange("p (h c) -> p h c", c=DK),
                    )
                    nc.gpsimd.memset(v5[:, :, DK:DK + 1], 1.0)

            XN = [xnpool.tile([P, S], f16, tag=f"XN{i}", name=f"XN{i}")
                  for i in range(2)]

            # ---------------- attention ----------------
            with tc.tile_pool(name="scps", bufs=2, space="PSUM") as scps, \
                 tc.tile_pool(name="pvps", bufs=1, space="PSUM") as pvps:
                def kb_front(qg, kb):
                    """mask DMA + scores + exp + mask-mul for one key block;
                    returns the two pa views (one per head pair)."""
                    mk = mpool.tile([P, QG], f16, tag="mk", name="mk")
                    nc.sync.dma_start(
                        mk[:],
                        maskT[kb * P:(kb + 1) * P, qg * QG:(qg + 1) * QG],
                    )
                    pa = ppool.tile([P, 4 * QG], f16, tag="pa", name="pa")
                    for pr in range(2):
                        sc = scps.tile([P, 2 * QG], f32, tag="sc", name="sc")
                        for hh in range(2):
                            h = pr * 2 + hh
                            kc, sub = divmod(h, 2)
                            nc.tensor.matmul(
                                sc[:, hh * QG:(hh + 1) * QG],
                                KT[kc][sub * DK:(sub + 1) * DK,
                                       kb * P:(kb + 1) * P],
                                QT[kc][sub * DK:(sub + 1) * DK,
                                       qg * QG:(qg + 1) * QG],
                                start=True, stop=True,
                                tile_position=(sub * DK, 0),
                            )
                        nc.scalar.activation(
                            pa[:, pr * 2 * QG:(pr + 1) * 2 * QG], sc[:],
                            AF.Exp, scale=float(SCALE))
                    pav4 = pa[:].rearrange("p (h n) -> p h n", n=QG)
                    nc.vector.tensor_tensor(
                        pav4, pav4,
                        mk[:].unsqueeze(1).broadcast_to((P, HPC, QG)),
                        op=OP.mult,
                    )
                    return pav4

                def kb_pv(pvt, kb, pav4):
                    for h in range(HPC):
                        nc.tensor.matmul(
                            pvt[h][:],
                            Vt[kb][:].rearrange(
                                "p (h c) -> p h c", c=DK + 1)[:, h, :],
                            pav4[:, h, :],
                            start=(kb == 0), stop=(kb == NKB - 1),
                        )

                def emit_outproj(qg):
                    for dm in range(D // P):
                        ops_t = pvps.tile([P, QG], f32, tag=f"pv{dm % HPC}",
                                          name=f"op{dm}")
                        for hd in range(2):
                            nc.tensor.matmul(
                                ops_t[:],
                                wo_sb[hd][:, dm * P:(dm + 1) * P],
                                XN[hd][:, qg * QG:(qg + 1) * QG],
                                start=(hd == 0), stop=(hd == 1),
                            )
                        ost = opool.tile([P, QG], f32, tag="ost", name="ost")
                        if dm % 2 == 0:
                            nc.scalar.activation(ost[:], ops_t[:], AF.Copy)
                        else:
                            nc.vector.tensor_copy(ost[:], ops_t[:])
                        nc.sync.dma_start(
                            outT[dm * P:(dm + 1) * P, qg * QG:(qg + 1) * QG],
                            ost[:])

                for qg in range(NQG):
                    pvt = [pvps.tile([DK + 1, QG], f32, tag=f"pv{h}",
                                     name=f"pv{h}") for h in range(HPC)]
                    for kb in range(NKB):
                        kb_pv(pvt, kb, kb_front(qg, kb))
                    # drain PV psum early, then normalize off-critical-path
                    # (DVE partition-offset writes must be 32-aligned, so
                    # the 4 denominator rows are gathered at 32-stride.)
                    XS = []
                    dn4 = npool1.tile([3 * 32 + 1, QG], f32, tag="dn4",
                                      name="dn4")
                    for h in range(HPC):
                        xs = xspool.tile([DK + 1, QG], f32, tag="xs",
                                         name="xs")
                        nc.scalar.activation(xs[:], pvt[h][:], AF.Copy)
                        XS.append(xs)
                    for h in range(HPC):
                        nc.vector.tensor_copy(dn4[32 * h:32 * h + 1, :],
                                              XS[h][DK:DK + 1, :])
                    rc4 = npool1.tile([3 * 32 + 1, QG], f32, tag="rc4",
                                      name="rc4")
                    nc.vector.reciprocal_approx_fast(rc4[:], dn4[:])
                    for h in range(HPC):
                        kc, sub = divmod(h, 2)
                        r1 = npool.tile([1, QG], f32, tag="r1", name="r1")
                        nc.vector.tensor_copy(r1[:],
                                              rc4[32 * h:32 * h + 1, :])
                        rb = npool.tile([DK, QG], f32, tag="rb", name="rb")
                        nc.gpsimd.partition_broadcast(rb[:], r1[:],
                                                      channels=DK)
                        nc.vector.tensor_tensor(
                            XN[kc][sub * DK:(sub + 1) * DK,
                                   qg * QG:(qg + 1) * QG],
                            XS[h][0:DK, :], rb[:], op=OP.mult,
                        )
                    emit_outproj(qg)

    nc.compile()
    return nc


def _get_nc():
    if "nc" not in _CACHE:
        _CACHE["nc"] = _build_nc()
    return _CACHE["nc"]


def _install_trace_shim():
    """Register the axon NTFF profile hook (dev/test only)."""
    import sys, types
    if "antenv.axon_hooks" in sys.modules:
        return
    try:
        import antenv
        from trn_agent_boot.trn_boot import _ntff_profile_via_ctypes
    except ImportError:
        return
    mod = types.ModuleType("antenv.axon_hooks")
    _hook = [_ntff_profile_via_ctypes("/opt/axon/libaxon_pjrt.so")]
    mod.get_axon_ntff_profile_hook = lambda: _hook[0]
    mod.set_axon_ntff_profile_hook = lambda h: _hook.__setitem__(0, h)
    sys.modules["antenv.axon_hooks"] = mod
    antenv.axon_hooks = mod


def kernel(query, key, value, mask, Wq, bq, Wk, bk, Wv, bv, Wo, bo):
    from concourse.bass_utils import run_bass_kernel_spmd

    query = np.asarray(query, np.float32)
    key = np.asarray(key, np.float32)
    value = np.asarray(value, np.float32)
    mask = np.asarray(mask)
    Wq = np.asarray(Wq, np.float32); bq = np.asarray(bq, np.float32)
    Wk = np.asarray(Wk, np.float32); bk = np.asarray(bk, np.float32)
    Wv = np.asarray(Wv, np.float32); bv = np.asarray(bv, np.float32)
    Wo = np.asarray(Wo, np.float32); bo = np.asarray(bo, np.float32)

    nc = _get_nc()

    qT = {b: np.ascontiguousarray(query[b].T.astype(np.float16))
          for b in range(B)}
    kT = {b: np.ascontiguousarray(key[b].T.astype(np.float16))
          for b in range(B)}
    vT = {b: np.ascontiguousarray(value[b].T.astype(np.float16))
          for b in range(B)}
    mT = {b: np.ascontiguousarray((mask[b].T == 0).astype(np.float16))
          for b in range(B)}

    in_maps = []
    for c in range(NCORES):
        b, hg = divmod(c, CPB)
        sl = slice(hg * HD, (hg + 1) * HD)
        in_maps.append({
            "xqT": qT[b],
            "xkT": kT[b],
            "xvT": vT[b],
            "wqT": np.ascontiguousarray(Wq[sl].T.astype(np.float16)),
            "wkT": np.ascontiguousarray(Wk[sl].T.astype(np.float16)),
            "wvT": np.ascontiguousarray(Wv[sl].T.astype(np.float16)),
            "woT": np.ascontiguousarray(Wo[:, sl].T.astype(np.float16)),
            "bq": np.ascontiguousarray(bq[sl].reshape(HD, 1)),
            "bk": np.ascontiguousarray(bk[sl].reshape(HD, 1)),
            "maskT": mT[b],
        })

    trace = bool(int(os.environ.get("BASS_KERNEL_TRACE", "0")))
    if trace:
        _install_trace_shim()
    res = run_bass_kernel_spmd(nc, in_maps, core_ids=list(range(NCORES)),
                               trace=trace)
    _CACHE["last_perf"] = res

    out = np.zeros((B, S, D), np.float32)
    for c in range(NCORES):
        b = c // CPB
        out[b] += res.results[c]["outT"].T
    out += (Wo @ bv + bo)[None, None, :]
    return out


# revision 22
# speedup vs baseline: 1.0436x; 1.0090x over previous
"""Multi-head attention on 8 Trainium2 NeuronCores.

Sharding: data-parallel over batch (2) x tensor-parallel over heads
(16 heads -> 4 per core). Each core computes QKV projections for its
4 heads, masked softmax attention, and a partial output projection
(row-parallel Wo); the host sums the 4 per-batch partials and folds
in the biases that commute with the linear output projection
(out[b] = sum_partials.T + bo + Wo @ bv).

Per-core pipeline (layouts chosen so no on-chip transposes are needed):
  - host sends query^T/key^T/value^T [D,S] fp16, per-core weight
    slices pre-transposed fp16, mask01^T fp16 (1=keep, 0=masked).
  - Q^T,K^T [256,S] = Wq/k^T.T @ x^T   (dout on partitions, fp16+FWL,
    bias added on ScalarE Identity-activation, fp16 out)
  - V [S,256] = x^T.T @ Wv^T           (tokens on partitions, fp16)
  - scores^T [k,q] = K^T_h.T @ Q^T_h   (two heads row-packed per PSUM tile)
  - p^T = exp(scores/8)                (ScalarE, fp16 out)
  - p^T *= mask01^T                    (DVE fp16 2x, broadcast across heads)
  - X'^T [65,q] = [V_h | 1].T @ p^T    (row 64 = softmax denominator)
  - drain X' to SBUF early (frees PSUM), then batched fast-reciprocal
    of the 4 denominators, GPSIMD partition-broadcast, DVE normalize.
  - out^T partial [D,S] = Wo^T.T @ X^T (fp16, f32 out, drained on ScalarE)
"""

import os
import numpy as np

B, S, D = 2, 2048, 1024
H, DK = 16, 64
NCORES = 8
CPB = 4                 # cores per batch
HPC = H // CPB          # 4 heads per core
HD = HPC * DK           # 256
P = 128
QG = 512                # query block
NQG = S // QG
NKB = S // P            # key blocks
NDC = D // P            # d_in chunks
SCALE = 1.0 / np.sqrt(DK)

_CACHE = {}


def _build_nc():
    import concourse.mybir as mybir
    import concourse.tile as tile
    from concourse import bacc

    f32 = mybir.dt.float32
    f16 = mybir.dt.float16
    AF = mybir.ActivationFunctionType
    OP = mybir.AluOpType

    nc = bacc.Bacc("TRN2", target_bir_lowering=False, debug=False)
    xqT = nc.dram_tensor("xqT", [D, S], f16, kind="ExternalInput")
    xkT = nc.dram_tensor("xkT", [D, S], f16, kind="ExternalInput")
    xvT = nc.dram_tensor("xvT", [D, S], f16, kind="ExternalInput")
    wqT = nc.dram_tensor("wqT", [D, HD], f16, kind="ExternalInput")
    wkT = nc.dram_tensor("wkT", [D, HD], f16, kind="ExternalInput")
    wvT = nc.dram_tensor("wvT", [D, HD], f16, kind="ExternalInput")
    woT = nc.dram_tensor("woT", [HD, D], f16, kind="ExternalInput")
    bq = nc.dram_tensor("bq", [HD, 1], f32, kind="ExternalInput")
    bk = nc.dram_tensor("bk", [HD, 1], f32, kind="ExternalInput")
    maskT = nc.dram_tensor("maskT", [S, S], f16, kind="ExternalInput")
    outT = nc.dram_tensor("outT", [D, S], f32, kind="ExternalOutput")

    with tile.TileContext(nc) as tc:
        with (
            tc.tile_pool(name="wpool", bufs=1) as wpool,
            tc.tile_pool(name="xkq", bufs=8) as xkq,
            tc.tile_pool(name="xqp", bufs=8) as xqp,
            tc.tile_pool(name="xvp", bufs=8) as xvp,
            tc.tile_pool(name="proj", bufs=1) as proj,
            tc.tile_pool(name="vpool", bufs=1) as vpool,
            tc.tile_pool(name="mpool", bufs=4) as mpool,
            tc.tile_pool(name="ppool", bufs=3) as ppool,
            tc.tile_pool(name="xspool", bufs=4) as xspool,
            tc.tile_pool(name="npool", bufs=2) as npool,
            tc.tile_pool(name="npool1", bufs=1) as npool1,
            tc.tile_pool(name="xnpool", bufs=1) as xnpool,
            tc.tile_pool(name="opool", bufs=4) as opool,
        ):
            # ---------------- weights -> SBUF ----------------
            wq_sb = wpool.tile([P, NDC * HD], f16, tag="wq")
            wk_sb = wpool.tile([P, NDC * HD], f16, tag="wk")
            wv_sb = wpool.tile([P, NDC * HD], f16, tag="wv")
            def _load_w(wsb, wdr):
                nc.sync.dma_start(
                    wsb[:].rearrange("p (c n) -> p c n", n=HD),
                    wdr.rearrange("(c p) n -> p c n", p=P),
                )
            _load_w(wk_sb, wkT)
            bq_sb = [wpool.tile([P, 1], f32, tag=f"bq{i}", name=f"bqt{i}")
                     for i in range(2)]
            bk_sb = [wpool.tile([P, 1], f32, tag=f"bk{i}", name=f"bkt{i}")
                     for i in range(2)]
            for i in range(2):
                nc.sync.dma_start(bq_sb[i][:], bq[i * P:(i + 1) * P, :])
                nc.sync.dma_start(bk_sb[i][:], bk[i * P:(i + 1) * P, :])
            wo_sb = [wpool.tile([P, D], f16, tag=f"wo{i}", name=f"wo{i}")
                     for i in range(2)]

            KT = [proj.tile([P, S], f16, tag=f"KT{i}", name=f"KT{i}")
                  for i in range(2)]
            QT = [proj.tile([P, S], f16, tag=f"QT{i}", name=f"QT{i}")
                  for i in range(2)]
            Vt = [vpool.tile([P, HPC * (DK + 1)], f16, tag=f"V{t}",
                             name=f"Vt{t}") for t in range(NKB)]

            with tc.tile_pool(name="pps", bufs=8, space="PSUM") as pps:
                # ---------------- K projection (c-outer, DMA-paced) ------
                xk = [xkq.tile([P, S], f16, tag="xin", name="xk")
                      for _ in range(NDC)]
                for c in range(NDC):
                    nc.sync.dma_start(xk[c][:], xkT[c * P:(c + 1) * P, :])
                ps_k = [pps.tile([P, QG], f32, tag="projps", name="pps")
                        for _ in range(2 * NQG)]
                for c in range(NDC):
                    for tg in range(NQG):
                        for kc in range(2):
                            nc.tensor.matmul(
                                ps_k[tg * 2 + kc][:],
                                wk_sb[:, c * HD + kc * P: c * HD + (kc + 1) * P],
                                xk[c][:, tg * QG:(tg + 1) * QG],
                                start=(c == 0), stop=(c == NDC - 1),
                            )
                for tg in range(NQG):
                    for kc in range(2):
                        nc.scalar.activation(
                            KT[kc][:, tg * QG:(tg + 1) * QG],
                            ps_k[tg * 2 + kc][:],
                            AF.Identity, bias=bk_sb[kc][:, 0:1],
                        )

                # ---------------- Q projection (c-outer) ----------------
                _load_w(wq_sb, wqT)
                xq = [xqp.tile([P, S], f16, tag="xq", name="xq")
                      for _ in range(NDC)]
                for c in range(NDC):
                    nc.sync.dma_start(xq[c][:], xqT[c * P:(c + 1) * P, :])
                ps_q = [pps.tile([P, QG], f32, tag="projps", name="pps")
                        for _ in range(2 * NQG)]
                for c in range(NDC):
                    for tg in range(NQG):
                        for kc in range(2):
                            nc.tensor.matmul(
                                ps_q[tg * 2 + kc][:],
                                wq_sb[:, c * HD + kc * P: c * HD + (kc + 1) * P],
                                xq[c][:, tg * QG:(tg + 1) * QG],
                                start=(c == 0), stop=(c == NDC - 1),
                            )
                for tg in range(NQG):
                    for kc in range(2):
                        nc.scalar.activation(
                            QT[kc][:, tg * QG:(tg + 1) * QG],
                            ps_q[tg * 2 + kc][:],
                            AF.Identity, bias=bq_sb[kc][:, 0:1],
                        )

            with tc.tile_pool(name="vps", bufs=2, space="PSUM") as vps:
                # ---------------- V projection ----------------
                _load_w(wv_sb, wvT)
                xv = [xvp.tile([P, S], f16, tag="xv", name="xv")
                      for _ in range(NDC)]
                for c in range(NDC):
                    nc.sync.dma_start(xv[c][:], xvT[c * P:(c + 1) * P, :])
                for i in range(2):
                    nc.sync.dma_start(wo_sb[i][:], woT[i * P:(i + 1) * P, :])
                for t in range(NKB):
                    vp = vps.tile([P, HD], f32, tag="vps", name="vps")
                    for c in range(NDC):
                        nc.tensor.matmul(
                            vp[:], xv[c][:, t * P:(t + 1) * P],
                            wv_sb[:, c * HD:(c + 1) * HD],
                            start=(c == 0), stop=(c == NDC - 1),
                        )
                    v5 = Vt[t][:].rearrange("p (h c) -> p h c", c=DK + 1)
                    nc.vector.tensor_copy(
                        v5[:, :, 0:DK],
                        vp[:].rearrange("p (h c) -> p h c", c=DK),
                    )
                    nc.gpsimd.memset(v5[:, :, DK:DK + 1], 1.0)

            XN = [xnpool.tile([P, S], f16, tag=f"XN{i}", name=f"XN{i}")
                  for i in range(2)]

            # ---------------- attention ----------------
            with tc.tile_pool(name="scps", bufs=2, space="PSUM") as scps, \
                 tc.tile_pool(name="pvps", bufs=1, space="PSUM") as pvps:
                mk2_hold = [None]

                def kb_front(qg, kb):
                    """scores + exp + mask-mul for one key block; the mask
                    for two key blocks is loaded by one DMA (3D AP over two
                    row blocks of maskT)."""
                    if kb % 2 == 0:
                        mk2 = mpool.tile([P, 2, QG], f16, tag="mk", name="mk")
                        nc.sync.dma_start(
                            mk2[:],
                            maskT.rearrange("(t p) n -> p t n", p=P)[
                                :, kb:kb + 2, qg * QG:(qg + 1) * QG],
                        )
                        mk2_hold[0] = mk2
                    mk = mk2_hold[0][:, kb % 2, :]
                    pa = ppool.tile([P, 4 * QG], f16, tag="pa", name="pa")
                    for pr in range(2):
                        sc = scps.tile([P, 2 * QG], f32, tag="sc", name="sc")
                        for hh in range(2):
                            h = pr * 2 + hh
                            kc, sub = divmod(h, 2)
                            nc.tensor.matmul(
                                sc[:, hh * QG:(hh + 1) * QG],
                                KT[kc][sub * DK:(sub + 1) * DK,
                                       kb * P:(kb + 1) * P],
                                QT[kc][sub * DK:(sub + 1) * DK,
                                       qg * QG:(qg + 1) * QG],
                                start=True, stop=True,
                                tile_position=(sub * DK, 0),
                            )
                        nc.scalar.activation(
                            pa[:, pr * 2 * QG:(pr + 1) * 2 * QG], sc[:],
                            AF.Exp, scale=float(SCALE))
                    pav4 = pa[:].rearrange("p (h n) -> p h n", n=QG)
                    nc.vector.tensor_tensor(
                        pav4, pav4,
                        mk.unsqueeze(1).broadcast_to((P, HPC, QG)),
                        op=OP.mult,
                    )
                    return pav4

                def kb_pv(pvt, kb, pav4):
                    for h in range(HPC):
                        nc.tensor.matmul(
                            pvt[h][:],
                            Vt[kb][:].rearrange(
                                "p (h c) -> p h c", c=DK + 1)[:, h, :],
                            pav4[:, h, :],
                            start=(kb == 0), stop=(kb == NKB - 1),
                        )

                def emit_outproj(qg):
                    for dm in range(D // P):
                        ops_t = pvps.tile([P, QG], f32, tag=f"pv{dm % HPC}",
                                          name=f"op{dm}")
                        for hd in range(2):
                            nc.tensor.matmul(
                                ops_t[:],
                                wo_sb[hd][:, dm * P:(dm + 1) * P],
                                XN[hd][:, qg * QG:(qg + 1) * QG],
                                start=(hd == 0), stop=(hd == 1),
                            )
                        ost = opool.tile([P, QG], f32, tag="ost", name="ost")
                        if dm % 2 == 0:
                            nc.scalar.activation(ost[:], ops_t[:], AF.Copy)
                        else:
                            nc.vector.tensor_copy(ost[:], ops_t[:])
                        nc.sync.dma_start(
                            outT[dm * P:(dm + 1) * P, qg * QG:(qg + 1) * QG],
                            ost[:])

                for qg in range(NQG):
                    pvt = [pvps.tile([DK + 1, QG], f32, tag=f"pv{h}",
                                     name=f"pv{h}") for h in range(HPC)]
                    for kb in range(NKB):
                        kb_pv(pvt, kb, kb_front(qg, kb))
                    # drain PV psum early, then normalize off-critical-path
                    # (DVE partition-offset writes must be 32-aligned, so
                    # the 4 denominator rows are gathered at 32-stride.)
                    XS = []
                    dn4 = npool1.tile([3 * 32 + 1, QG], f32, tag="dn4",
                                      name="dn4")
                    for h in range(HPC):
                        xs = xspool.tile([DK + 1, QG], f32, tag="xs",
                                         name="xs")
                        nc.scalar.activation(xs[:], pvt[h][:], AF.Copy)
                        XS.append(xs)
                    for h in range(HPC):
                        nc.vector.tensor_copy(dn4[32 * h:32 * h + 1, :],
                                              XS[h][DK:DK + 1, :])
                    rc4 = npool1.tile([3 * 32 + 1, QG], f32, tag="rc4",
                                      name="rc4")
                    nc.vector.reciprocal_approx_fast(rc4[:], dn4[:])
                    for h in range(HPC):
                        kc, sub = divmod(h, 2)
                        r1 = npool.tile([1, QG], f32, tag="r1", name="r1")
                        nc.vector.tensor_copy(r1[:],
                                              rc4[32 * h:32 * h + 1, :])
                        rb = npool.tile([DK, QG], f32, tag="rb", name="rb")
                        nc.gpsimd.partition_broadcast(rb[:], r1[:],
                                                      channels=DK)
                        nc.vector.tensor_tensor(
                            XN[kc][sub * DK:(sub + 1) * DK,
                                   qg * QG:(qg + 1) * QG],
                            XS[h][0:DK, :], rb[:], op=OP.mult,
                        )
                    emit_outproj(qg)

    nc.compile()
    return nc


def _get_nc():
    if "nc" not in _CACHE:
        _CACHE["nc"] = _build_nc()
    return _CACHE["nc"]


def _install_trace_shim():
    """Register the axon NTFF profile hook (dev/test only)."""
    import sys, types
    if "antenv.axon_hooks" in sys.modules:
        return
    try:
        import antenv
        from trn_agent_boot.trn_boot import _ntff_profile_via_ctypes
    except ImportError:
        return
    mod = types.ModuleType("antenv.axon_hooks")
    _hook = [_ntff_profile_via_ctypes("/opt/axon/libaxon_pjrt.so")]
    mod.get_axon_ntff_profile_hook = lambda: _hook[0]
    mod.set_axon_ntff_profile_hook = lambda h: _hook.__setitem__(0, h)
    sys.modules["antenv.axon_hooks"] = mod
    antenv.axon_hooks = mod


def kernel(query, key, value, mask, Wq, bq, Wk, bk, Wv, bv, Wo, bo):
    from concourse.bass_utils import run_bass_kernel_spmd

    query = np.asarray(query, np.float32)
    key = np.asarray(key, np.float32)
    value = np.asarray(value, np.float32)
    mask = np.asarray(mask)
    Wq = np.asarray(Wq, np.float32); bq = np.asarray(bq, np.float32)
    Wk = np.asarray(Wk, np.float32); bk = np.asarray(bk, np.float32)
    Wv = np.asarray(Wv, np.float32); bv = np.asarray(bv, np.float32)
    Wo = np.asarray(Wo, np.float32); bo = np.asarray(bo, np.float32)

    nc = _get_nc()

    qT = {b: np.ascontiguousarray(query[b].T.astype(np.float16))
          for b in range(B)}
    kT = {b: np.ascontiguousarray(key[b].T.astype(np.float16))
          for b in range(B)}
    vT = {b: np.ascontiguousarray(value[b].T.astype(np.float16))
          for b in range(B)}
    mT = {b: np.ascontiguousarray((mask[b].T == 0).astype(np.float16))
          for b in range(B)}

    in_maps = []
    for c in range(NCORES):
        b, hg = divmod(c, CPB)
        sl = slice(hg * HD, (hg + 1) * HD)
        in_maps.append({
            "xqT": qT[b],
            "xkT": kT[b],
            "xvT": vT[b],
            "wqT": np.ascontiguousarray(Wq[sl].T.astype(np.float16)),
            "wkT": np.ascontiguousarray(Wk[sl].T.astype(np.float16)),
            "wvT": np.ascontiguousarray(Wv[sl].T.astype(np.float16)),
            "woT": np.ascontiguousarray(Wo[:, sl].T.astype(np.float16)),
            "bq": np.ascontiguousarray(bq[sl].reshape(HD, 1)),
            "bk": np.ascontiguousarray(bk[sl].reshape(HD, 1)),
            "maskT": mT[b],
        })

    trace = bool(int(os.environ.get("BASS_KERNEL_TRACE", "0")))
    if trace:
        _install_trace_shim()
    res = run_bass_kernel_spmd(nc, in_maps, core_ids=list(range(NCORES)),
                               trace=trace)
    _CACHE["last_perf"] = res

    out = np.zeros((B, S, D), np.float32)
    for c in range(NCORES):
        b = c // CPB
        out[b] += res.results[c]["outT"].T
    out += (Wo @ bv + bo)[None, None, :]
    return out
